# revision 9
# baseline (speedup 1.0000x reference)
"""Trainium2 Bass kernel for a dense transformer encoder layer (v2).

Reference semantics (B=2, S=2048, D=1024, H=16, DH=64, HID=4096):
    q = einsum('bsd,hde->bhse', x, Wq) + bq          (q == k == v, source bug)
    prob = softmax(q @ q^T / sqrt(DH))
    attn = concat_heads(prob @ q)
    x1 = LN(x + attn);  ff = relu(x1 @ W1 + b1) @ W2 + b2;  out = LN(x1 + ff)

Sharding: 8 cores, core c -> batch b=c//4, token quarter t=c%4.  The host
rotates x[b] so the core's 512 queries are tokens 0:511 (attention is
permutation-equivariant over keys); q/k/v for the full sequence are computed
on every core of the 4-core group (zero collectives).

v2 design (driven by the TimelineSim cost model):
- fp8e4m3 DoubleRow matmuls (two stacked 128-row K-subtiles per instruction)
  for the q projection, attention@V, and FFN W1.  Scores and W2 stay bf16
  for accuracy.  All weight layouts are prepared host-side; there are no
  DMA transposes and no DRAM round-trips for intermediates.
- q is computed in BOTH layouts directly on the PE: feature-major qT (bf16,
  for scores) and token-major qa8 (fp8, for attention@V), reusing the same
  x^T / Wq operand tensors in swapped stationary/moving roles.
- bq is folded out of the value path: softmax weights sum to 1, so
  attn = P@(x Wq) + bq, and the +bq folds into the host residual x_q=x+bq.
  qT keeps the bias (scores need it).
- attention@V emits token-major [128q, 65] psum tiles (64 head features +
  the softmax denominator from an all-ones column of qa8); the epilogue is
  a reciprocal + one fused multiply-add into y1 per head - no transposes.
- Softmax exp on ACT is the critical engine (~127us); attention runs
  query-block-major and each block's FFN (token-major outputs too) is
  emitted under the exp shadow via a work-unit FIFO.
"""

import numpy as np

import concourse.bacc as bacc
import concourse.mybir as mybir
from concourse import tile
from concourse.bass_utils import run_bass_kernel_spmd

dt = mybir.dt
AF = mybir.ActivationFunctionType
ALU = mybir.AluOpType
DR = mybir.MatmulPerfMode.DoubleRow

B, S, D = 2, 2048, 1024
H, DH, HID = 16, 64, 4096
SQ = S // 4            # tokens per core
NCORES = 8
EPS = 1e-5
F32, BF16, FP8 = dt.float32, dt.bfloat16, dt.float8e4

_BUILD_CACHE = {}


def _build(apply_affine: bool):
    if apply_affine in _BUILD_CACHE:
        return _BUILD_CACHE[apply_affine]

    nc = bacc.Bacc("TRN2", target_bir_lowering=False, debug=False,
                   num_devices=NCORES)

    xt8_d = nc.dram_tensor("xt8", [128, 16384], FP8, kind="ExternalInput").ap()
    wq8_d = nc.dram_tensor("wq8", [128, 8192], FP8, kind="ExternalInput").ap()
    bqT_d = nc.dram_tensor("bqT", [128, 8], F32, kind="ExternalInput").ap()
    xq_d = nc.dram_tensor("xq", [SQ, D], F32, kind="ExternalInput").ap()
    w18_d = nc.dram_tensor("w18", [128, 32768], FP8,
                           kind="ExternalInput").ap()
    b1T_d = nc.dram_tensor("b1T", [128, 32], F32, kind="ExternalInput").ap()
    w2_d = nc.dram_tensor("w2", [HID, D], BF16, kind="ExternalInput").ap()
    b2bc_d = nc.dram_tensor("b2bc", [128, D], F32, kind="ExternalInput").ap()
    if apply_affine:
        g1_d = nc.dram_tensor("g1bc", [128, D], F32, kind="ExternalInput").ap()
        be1_d = nc.dram_tensor("be1bc", [128, D], F32,
                               kind="ExternalInput").ap()
        g2_d = nc.dram_tensor("g2bc", [128, D], F32, kind="ExternalInput").ap()
        be2_d = nc.dram_tensor("be2bc", [128, D], F32,
                               kind="ExternalInput").ap()
    out_q = nc.dram_tensor("out_q", [SQ, D], F32, kind="ExternalOutput").ap()

    with tile.TileContext(nc) as tc:
        with (
            tc.tile_pool(name="const", bufs=1) as cpool,
            tc.tile_pool(name="data", bufs=1) as dpool,
        ):
            # ---- constants ----
            bqT = cpool.tile([128, 8], F32)
            nc.sync.dma_start(bqT[:], bqT_d[:])
            b1T = cpool.tile([128, 32], F32)
            nc.sync.dma_start(b1T[:], b1T_d[:])
            b2bc = cpool.tile([128, D], F32)
            nc.gpsimd.dma_start(b2bc[:], b2bc_d[:])
            if apply_affine:
                g1bc = cpool.tile([128, D], F32)
                nc.gpsimd.dma_start(g1bc[:], g1_d[:])
                be1bc = cpool.tile([128, D], F32)
                nc.gpsimd.dma_start(be1bc[:], be1_d[:])
                g2bc = cpool.tile([128, D], F32)
                nc.gpsimd.dma_start(g2bc[:], g2_d[:])
                be2bc = cpool.tile([128, D], F32)
                nc.gpsimd.dma_start(be2bc[:], be2_d[:])
            eps_sb = cpool.tile([128, 1], F32)
            nc.vector.memset(eps_sb[:], EPS)

            col_i = cpool.tile([128, 128], F32)
            nc.gpsimd.iota(col_i[:], [[1, 128]], channel_multiplier=0,
                           allow_small_or_imprecise_dtypes=True)
            row_i = cpool.tile([128, 1], F32)
            nc.gpsimd.iota(row_i[:], [[0, 1]], channel_multiplier=1,
                           allow_small_or_imprecise_dtypes=True)
            idn = cpool.tile([128, 128], BF16)
            nc.vector.tensor_scalar(idn[:], col_i[:], row_i[:, 0:1], None,
                                    ALU.is_equal)

            # ---- persistent data tiles ----
            y1 = []
            for qb in range(4):
                t = dpool.tile([128, D], F32, tag=f"y1_{qb}", name=f"y1_{qb}")
                nc.sync.dma_start(t[:], xq_d[qb * 128:(qb + 1) * 128, :])
                y1.append(t)
            W18 = dpool.tile([128, 8, HID], FP8, tag="w18")
            for part in range(4):
                nc.gpsimd.dma_start(
                    W18[:, 2 * part:2 * part + 2, :],
                    w18_d[:, part * 8192:(part + 1) * 8192]
                    .rearrange("p (j m) -> p j m", j=2))
            w2sb = []
            for j2 in range(32):
                t = dpool.tile([128, D], BF16, tag=f"w2_{j2}", name=f"w2_{j2}")
                eng = (nc.sync, nc.gpsimd)[j2 % 2]
                eng.dma_start(t[:], w2_d[j2 * 128:(j2 + 1) * 128, :])
                w2sb.append(t)
            qT = [dpool.tile([128, S], BF16, tag=f"qT{p}", name=f"qT{p}")
                  for p in range(8)]
            # qa8[c][p, j, pp*130 + hh*65 + e] = q_nat[key 128*(2c+j)+p,
            #   feature pp*128+hh*64+e]; column e=64 of each 65-block is 1.0
            qa8 = [dpool.tile([128, 2, 1040], FP8, tag=f"qa8_{c}",
                              name=f"qa8_{c}") for c in range(8)]
            for c in range(8):
                nc.gpsimd.memset(
                    qa8[c][:].rearrange("p j (pp hh e) -> p (j pp hh) e",
                                        pp=8, hh=2)[:, :, 64:65], 1.0)

            # ---- phase B: q projection, both layouts ----
            with (
                tc.tile_pool(name="bload", bufs=1) as bpool,
                tc.tile_pool(name="qpps", bufs=2, space="PSUM") as qpps,
                tc.tile_pool(name="qnps", bufs=2, space="PSUM") as qnps,
            ):
                xT8 = []
                for jj in range(4):
                    t = bpool.tile([128, 2, S], FP8, tag=f"xT8_{jj}", name=f"xT8_{jj}")
                    nc.sync.dma_start(
                        t[:], xt8_d[:, jj * 4096:(jj + 1) * 4096]
                        .rearrange("p (j t) -> p j t", j=2))
                    xT8.append(t)
                wq8 = bpool.tile([128, 8, D], FP8, tag="wq8")
                nc.sync.dma_start(
                    wq8[:], wq8_d.rearrange("p (j m) -> p j m", j=8))

                # token-major projection first (qa8 ready before any wv;
                # ACT starts packing ~2us in)
                for tb in range(16):
                    ps = qnps.tile([128, D], F32, tag="qn", name=f"qn{tb}")
                    for hh in range(2):
                        for jj in range(4):
                            nc.tensor.matmul(
                                ps[:, hh * 512:(hh + 1) * 512],
                                xT8[jj][:, :, tb * 128:(tb + 1) * 128],
                                wq8[:, 2 * jj:2 * jj + 2,
                                    hh * 512:(hh + 1) * 512],
                                start=(jj == 0), stop=(jj == 3),
                                perf_mode=DR)
                    nc.scalar.copy(
                        qa8[tb // 2][:, tb % 2, :]
                        .rearrange("p (pp hh e) -> p pp hh e", pp=8, hh=2)
                        [:, :, :, 0:64],
                        ps[:].rearrange("p (pp hh e) -> p pp hh e",
                                        pp=8, hh=2))

                for p in range(8):
                    for n in range(4):
                        ps = qpps.tile([128, 512], F32, tag="qp",
                                       name=f"qp{p}_{n}")
                        for jj in range(4):
                            nc.tensor.matmul(
                                ps[:],
                                wq8[:, 2 * jj:2 * jj + 2,
                                    p * 128:(p + 1) * 128],
                                xT8[jj][:, :, n * 512:(n + 1) * 512],
                                start=(jj == 0), stop=(jj == 3),
                                perf_mode=DR)
                        nc.vector.tensor_scalar_add(
                            qT[p][:, n * 512:(n + 1) * 512], ps[:],
                            bqT[:, p:p + 1])

            # ---- phase C: attention with pipelined FFN ----
            with (
                tc.tile_pool(name="cdata", bufs=1) as cd,
                tc.tile_pool(name="scr", bufs=2) as spool,
                tc.tile_pool(name="out1", bufs=1) as opool,
                tc.tile_pool(name="scps", bufs=1, space="PSUM") as scps,
                tc.tile_pool(name="uvps", bufs=2, space="PSUM") as uvps,
                tc.tile_pool(name="fps", bufs=1, space="PSUM") as fps,
                tc.tile_pool(name="w2ps", bufs=1, space="PSUM") as w2ps,
                tc.tile_pool(name="esb", bufs=6) as epool,
            ):
                x1bf = [cd.tile([128, D], BF16, tag=f"x1bf{qb}",
                        name=f"x1bf{qb}") for qb in range(4)]
                x1t8 = cd.tile([128, 8, 256], FP8, tag="x1t8")
                h1all = cd.tile([128, 32, 256], BF16, tag="h1all")
                ps2_live = {}

                def ln_unit(qb):
                    def emit():
                        _layer_norm(nc, spool, y1[qb], x1bf[qb], eps_sb,
                                    (g1bc, be1bc) if apply_affine else None)
                    return emit

                def x1t_unit(qb):
                    def emit():
                        sl = qb % 2
                        with tc.tile_pool(name=f"tp{qb}", bufs=1,
                                          space="PSUM") as tps:
                            for g in range(2):
                                pst = tps.tile([128, 4, 128], BF16,
                                               tag="tps", name=f"tp{qb}_{g}")
                                for k4 in range(4):
                                    k = 4 * g + k4
                                    nc.tensor.transpose(
                                        pst[:, k4, :],
                                        x1bf[qb][:, k * 128:(k + 1) * 128],
                                        idn[:])
                                nc.vector.tensor_copy(
                                    x1t8[:, 4 * g:4 * g + 4,
                                         sl * 128:(sl + 1) * 128], pst[:])
                    return emit

                def w1_unit(qb, g):
                    def emit():
                        sl = qb % 2
                        ps = fps.tile([128, 512], F32, tag="w1p",
                                      name=f"w1_{qb}_{g}")
                        for j4 in range(4):
                            j2 = 4 * g + j4
                            for jj in range(4):
                                nc.tensor.matmul(
                                    ps[:, j4 * 128:(j4 + 1) * 128],
                                    W18[:, 2 * jj:2 * jj + 2,
                                        j2 * 128:(j2 + 1) * 128],
                                    x1t8[:, 2 * jj:2 * jj + 2,
                                         sl * 128:(sl + 1) * 128],
                                    start=(jj == 0), stop=(jj == 3),
                                    perf_mode=DR)
                        for j4 in range(4):
                            nc.vector.tensor_scalar(
                                h1all[:, 4 * g + j4,
                                      sl * 128:(sl + 1) * 128],
                                ps[:, j4 * 128:(j4 + 1) * 128],
                                b1T[:, 4 * g + j4:4 * g + j4 + 1], 0.0,
                                ALU.add, ALU.max)
                    return emit

                def w2_unit(tb, g):
                    def emit():
                        sl = tb % 2
                        if g == 0:
                            ps2_live[tb] = w2ps.tile(
                                [128, D], F32, tag="w2p", name=f"w2ps_{tb}")
                        ps2 = ps2_live[tb]
                        for j4 in range(4):
                            j2 = 4 * g + j4
                            for fc in range(2):
                                nc.tensor.matmul(
                                    ps2[:, fc * 512:(fc + 1) * 512],
                                    h1all[:, j2, sl * 128:(sl + 1) * 128],
                                    w2sb[j2][:, fc * 512:(fc + 1) * 512],
                                    start=(g == 0 and j4 == 0),
                                    stop=(g == 7 and j4 == 3))
                        if g == 7:
                            t1 = y1[tb]        # y1 is dead after LN1
                            nc.vector.tensor_tensor(
                                t1[:], ps2[:], x1bf[tb][:], ALU.add)
                            del ps2_live[tb]
                            y2 = spool.tile([128, D], F32, tag="y2", bufs=1,
                                            name=f"y2_{tb}")
                            nc.gpsimd.tensor_tensor(
                                y2[:], t1[:], b2bc[:], ALU.add)
                            x2 = opool.tile([128, D], F32, tag="x2",
                                            name=f"x2_{tb}")
                            _layer_norm(nc, spool, y2, x2, eps_sb,
                                        (g2bc, be2bc) if apply_affine
                                        else None)
                            nc.sync.dma_start(
                                out_q[tb * 128:(tb + 1) * 128, :], x2[:])
                    return emit

                ffn_fifo = []

                def fill():
                    if ffn_fifo:
                        ffn_fifo.pop(0)()

                def attn_pair(qb, p):
                    # scores+exp chunk by chunk, FFN filler between chunks
                    # so the in-order PE never parks on the sc psum WAR
                    E_subs = []
                    for sub in range(4):
                        half, kk = sub // 2, sub % 2
                        sc = scps.tile([128, 1024], F32, tag="sc",
                                       name=f"sc{qb}_{p}_{sub}")
                        for kbl in range(8):
                            kb = kk * 8 + kbl
                            nc.tensor.matmul(
                                sc[:, kbl * 128:(kbl + 1) * 128],
                                qT[p][half * 64:half * 64 + 64,
                                      kb * 128:(kb + 1) * 128],
                                qT[p][half * 64:half * 64 + 64,
                                      qb * 128:(qb + 1) * 128],
                                start=True, stop=True)
                        E = epool.tile([128, 1024], FP8, tag="E",
                                       name=f"E{qb}_{p}_{sub}")
                        nc.scalar.activation(E[:], sc[:], AF.Exp,
                                             scale=0.125)
                        E_subs.append(E)
                        fill()
                    uv = uvps.tile([128, 130], F32, tag="uv",
                                   name=f"uv{qb}_{p}")
                    for half in range(2):
                        off = p * 130 + half * 65
                        for c in range(8):
                            E = E_subs[half * 2 + c // 4]
                            cl = c % 4
                            nc.tensor.matmul(
                                uv[:, half * 65:half * 65 + 65],
                                E[:].rearrange("p (j q) -> p j q", j=8)
                                [:, 2 * cl:2 * cl + 2, :],
                                qa8[c][:, :, off:off + 65],
                                start=(c == 0), stop=(c == 7),
                                perf_mode=DR)
                    for half in range(2):
                        h = 2 * p + half
                        rcp = spool.tile([128, 1], F32, tag="rcp",
                                         bufs=4, name=f"rcp{qb}_{h}")
                        nc.vector.reciprocal(
                            rcp[:], uv[:, half * 65 + 64:half * 65 + 65])
                        nc.vector.scalar_tensor_tensor(
                            y1[qb][:, h * 64:(h + 1) * 64],
                            uv[:, half * 65:half * 65 + 64],
                            rcp[:, 0:1],
                            y1[qb][:, h * 64:(h + 1) * 64],
                            ALU.mult, ALU.add)
                    fill()

                for qb in range(4):
                    for p in range(8):
                        attn_pair(qb, p)
                    ffn_fifo.append(ln_unit(qb))
                    ffn_fifo.append(x1t_unit(qb))
                    for g in range(8):
                        ffn_fifo.append(w1_unit(qb, g))
                    for g in range(8):
                        ffn_fifo.append(w2_unit(qb, g))
                while ffn_fifo:
                    ffn_fifo.pop(0)()

    nc.compile()
    _BUILD_CACHE[apply_affine] = nc
    return nc


def _layer_norm(nc, pool, y, out, eps_sb, affine):
    """out = (y - mean) * rsqrt(var + EPS) [* g + b]; free-dim D, f32 in.

    The tensor_tensor_reduce product output is junk scratch; it is written
    into `out`, which is then overwritten by the real normalized value.
    """
    s1 = pool.tile([128, 1], F32, tag="ln_s1")
    nc.vector.reduce_sum(s1[:], y[:], axis=mybir.AxisListType.X)
    mean = pool.tile([128, 1], F32, tag="ln_mean")
    nc.vector.tensor_scalar_mul(mean[:], s1[:], 1.0 / D)
    sqs = pool.tile([128, 1], F32, tag="ln_sqs")
    nc.vector.tensor_tensor_reduce(out[:], y[:], y[:], 1.0 / D, 0.0,
                                   ALU.mult, ALU.add, sqs[:])
    msq = pool.tile([128, 1], F32, tag="ln_msq")
    nc.vector.tensor_tensor(msq[:], mean[:], mean[:], ALU.mult)
    var = pool.tile([128, 1], F32, tag="ln_var")
    nc.vector.tensor_tensor(var[:], sqs[:], msq[:], ALU.subtract)
    lnv = pool.tile([128, 1], F32, tag="ln_lnv")
    nc.scalar.activation(lnv[:], var[:], AF.Ln, bias=eps_sb[:, 0:1])
    rstd = pool.tile([128, 1], F32, tag="ln_rstd")
    nc.scalar.activation(rstd[:], lnv[:], AF.Exp, scale=-0.5)
    if affine is None:
        nc.vector.tensor_scalar(out[:], y[:], mean[:, 0:1], rstd[:, 0:1],
                                ALU.subtract, ALU.mult)
    else:
        g_bc, b_bc = affine
        nc.vector.tensor_scalar(out[:], y[:], mean[:, 0:1], rstd[:, 0:1],
                                ALU.subtract, ALU.mult)
        nc.vector.tensor_tensor(out[:], out[:], g_bc[:], ALU.mult)
        nc.vector.tensor_tensor(out[:], out[:], b_bc[:], ALU.add)


def kernel(x, Wq, bq, ln1_g, ln1_b, W1, b1, W2, b2, ln2_g, ln2_b):
    x = np.asarray(x, np.float32)
    bf = dt.np(BF16)
    f8 = dt.np(FP8)
    trivial = (np.all(ln1_g == 1) and np.all(ln1_b == 0)
               and np.all(ln2_g == 1) and np.all(ln2_b == 0))
    nc = _build(apply_affine=not trivial)

    WqF = np.asarray(Wq, np.float32).transpose(1, 0, 2).reshape(D, D)
    bqF = np.asarray(bq, np.float32).reshape(D)
    W1f = np.asarray(W1, np.float32)

    wq8 = np.ascontiguousarray(
        WqF.astype(f8).reshape(8, 128, D).transpose(1, 0, 2)
        .reshape(128, 8 * D))
    bqT = np.ascontiguousarray(bqF.reshape(8, 128).T)
    w18 = np.ascontiguousarray(
        W1f.astype(f8).reshape(8, 128, HID).transpose(1, 0, 2)
        .reshape(128, 8 * HID))
    b1T = np.ascontiguousarray(
        np.asarray(b1, np.float32).reshape(32, 128).T)
    w2bf = np.asarray(W2, np.float32).astype(bf)
    b2bc = np.ascontiguousarray(
        np.broadcast_to(np.asarray(b2, np.float32), (128, D)))

    base = {"wq8": wq8, "bqT": bqT, "w18": w18, "b1T": b1T,
            "w2": w2bf, "b2bc": b2bc}
    if not trivial:
        for name, v in (("g1bc", ln1_g), ("be1bc", ln1_b),
                        ("g2bc", ln2_g), ("be2bc", ln2_b)):
            base[name] = np.ascontiguousarray(
                np.broadcast_to(np.asarray(v, np.float32), (128, D)))

    in_maps = []
    for c in range(NCORES):
        b, t = divmod(c, 4)
        xb = np.concatenate([x[b, t * SQ:], x[b, :t * SQ]], axis=0)
        xt8 = np.ascontiguousarray(
            xb.T.astype(f8).reshape(4, 2, 128, S).transpose(2, 0, 1, 3)
            .reshape(128, 16384))
        in_maps.append({
            **base,
            "xt8": xt8,
            "xq": np.ascontiguousarray(xb[:SQ] + bqF[None, :]),
        })

    import os
    trace = bool(int(os.environ.get("KERNEL_TRACE", "0")))
    kw = {}
    if trace:
        kw = dict(trace=True,
                  tmpdir=os.environ.get("KERNEL_TRACE_DIR") or None)
    res = run_bass_kernel_spmd(nc, in_maps, core_ids=list(range(NCORES)),
                               **kw)
    if trace:
        print(f"HW exec time: {res.exec_time_ns} ns")
    out = np.empty((B, S, D), np.float32)
    for c in range(NCORES):
        b, t = divmod(c, 4)
        out[b, t * SQ:(t + 1) * SQ] = res.results[c]["out_q"]
    return out


# revision 11
# speedup vs baseline: 1.1355x; 1.1355x over previous
"""Trainium2 Bass kernel for a dense transformer encoder layer (v2).

Reference semantics (B=2, S=2048, D=1024, H=16, DH=64, HID=4096):
    q = einsum('bsd,hde->bhse', x, Wq) + bq          (q == k == v, source bug)
    prob = softmax(q @ q^T / sqrt(DH))
    attn = concat_heads(prob @ q)
    x1 = LN(x + attn);  ff = relu(x1 @ W1 + b1) @ W2 + b2;  out = LN(x1 + ff)

Sharding: 8 cores, core c -> batch b=c//4, token quarter t=c%4.  The host
rotates x[b] so the core's 512 queries are tokens 0:511 (attention is
permutation-equivariant over keys); q/k/v for the full sequence are computed
on every core of the 4-core group (zero collectives).

v2 design (driven by the TimelineSim cost model):
- fp8e4m3 DoubleRow matmuls (two stacked 128-row K-subtiles per instruction)
  for the q projection, attention@V, and FFN W1.  Scores and W2 stay bf16
  for accuracy.  All weight layouts are prepared host-side; there are no
  DMA transposes and no DRAM round-trips for intermediates.
- q is computed in BOTH layouts directly on the PE: feature-major qT (bf16,
  for scores) and token-major qa8 (fp8, for attention@V), reusing the same
  x^T / Wq operand tensors in swapped stationary/moving roles.
- bq is folded out of the value path: softmax weights sum to 1, so
  attn = P@(x Wq) + bq, and the +bq folds into the host residual x_q=x+bq.
  qT keeps the bias (scores need it).
- attention@V emits token-major [128q, 65] psum tiles (64 head features +
  the softmax denominator from an all-ones column of qa8); the epilogue is
  a reciprocal + one fused multiply-add into y1 per head - no transposes.
- Softmax exp on ACT is the critical engine (~127us); attention runs
  query-block-major and each block's FFN (token-major outputs too) is
  emitted under the exp shadow via a work-unit FIFO.
"""

import numpy as np

import concourse.bacc as bacc
import concourse.mybir as mybir
from concourse import tile
from concourse.bass_utils import run_bass_kernel_spmd

dt = mybir.dt
AF = mybir.ActivationFunctionType
ALU = mybir.AluOpType
DR = mybir.MatmulPerfMode.DoubleRow

B, S, D = 2, 2048, 1024
H, DH, HID = 16, 64, 4096
SQ = S // 4            # tokens per core
NCORES = 8
EPS = 1e-5
F32, BF16, FP8 = dt.float32, dt.bfloat16, dt.float8e4

_BUILD_CACHE = {}


def _build(apply_affine: bool):
    if apply_affine in _BUILD_CACHE:
        return _BUILD_CACHE[apply_affine]

    nc = bacc.Bacc("TRN2", target_bir_lowering=False, debug=False,
                   num_devices=NCORES)

    xt8_d = nc.dram_tensor("xt8", [128, 16384], FP8, kind="ExternalInput").ap()
    wq8_d = nc.dram_tensor("wq8", [128, 8192], FP8, kind="ExternalInput").ap()
    bqT_d = nc.dram_tensor("bqT", [128, 8], F32, kind="ExternalInput").ap()
    xq_d = nc.dram_tensor("xq", [SQ, D], F32, kind="ExternalInput").ap()
    w18_d = nc.dram_tensor("w18", [128, 32768], FP8,
                           kind="ExternalInput").ap()
    b1T_d = nc.dram_tensor("b1T", [128, 32], F32, kind="ExternalInput").ap()
    w2_d = nc.dram_tensor("w2", [HID, D], BF16, kind="ExternalInput").ap()
    b2bc_d = nc.dram_tensor("b2bc", [128, D], F32, kind="ExternalInput").ap()
    if apply_affine:
        g1_d = nc.dram_tensor("g1bc", [128, D], F32, kind="ExternalInput").ap()
        be1_d = nc.dram_tensor("be1bc", [128, D], F32,
                               kind="ExternalInput").ap()
        g2_d = nc.dram_tensor("g2bc", [128, D], F32, kind="ExternalInput").ap()
        be2_d = nc.dram_tensor("be2bc", [128, D], F32,
                               kind="ExternalInput").ap()
    out_q = nc.dram_tensor("out_q", [SQ, D], F32, kind="ExternalOutput").ap()

    with tile.TileContext(nc) as tc:
        with (
            tc.tile_pool(name="const", bufs=1) as cpool,
            tc.tile_pool(name="data", bufs=1) as dpool,
        ):
            qT = [dpool.tile([128, S], BF16, tag=f"qT{p}", name=f"qT{p}")
                  for p in range(8)]
            # qa8[c][p, j, pp*130 + hh*65 + e] = q_nat[key 128*(2c+j)+p,
            #   feature pp*128+hh*64+e]; column e=64 of each 65-block is 1.0
            qa8 = [dpool.tile([128, 2, 1040], FP8, tag=f"qa8_{c}",
                              name=f"qa8_{c}") for c in range(8)]
            for c in range(8):
                nc.gpsimd.memset(
                    qa8[c][:].rearrange("p j (pp hh e) -> p (j pp hh) e",
                                        pp=8, hh=2)[:, :, 64:65], 1.0)

            # ---- phase B: q projection, both layouts ----
            with (
                tc.tile_pool(name="bload", bufs=1) as bpool,
                tc.tile_pool(name="qpps", bufs=2, space="PSUM") as qpps,
                tc.tile_pool(name="qnps", bufs=2, space="PSUM") as qnps,
            ):
                xT8 = []
                for jj in range(4):
                    t = bpool.tile([128, 2, S], FP8, tag=f"xT8_{jj}", name=f"xT8_{jj}")
                    nc.sync.dma_start(
                        t[:], xt8_d[:, jj * 4096:(jj + 1) * 4096]
                        .rearrange("p (j t) -> p j t", j=2))
                    xT8.append(t)
                wq8 = bpool.tile([128, 8, D], FP8, tag="wq8")
                nc.sync.dma_start(
                    wq8[:], wq8_d.rearrange("p (j m) -> p j m", j=8))

                # ---- constants ----
                bqT = cpool.tile([128, 8], F32)
                nc.sync.dma_start(bqT[:], bqT_d[:])
                b1T = cpool.tile([128, 32], F32)
                nc.sync.dma_start(b1T[:], b1T_d[:])
                b2bc = cpool.tile([128, D], F32)
                nc.gpsimd.dma_start(b2bc[:], b2bc_d[:])
                if apply_affine:
                    g1bc = cpool.tile([128, D], F32)
                    nc.gpsimd.dma_start(g1bc[:], g1_d[:])
                    be1bc = cpool.tile([128, D], F32)
                    nc.gpsimd.dma_start(be1bc[:], be1_d[:])
                    g2bc = cpool.tile([128, D], F32)
                    nc.gpsimd.dma_start(g2bc[:], g2_d[:])
                    be2bc = cpool.tile([128, D], F32)
                    nc.gpsimd.dma_start(be2bc[:], be2_d[:])
                eps_sb = cpool.tile([128, 1], F32)
                nc.vector.memset(eps_sb[:], EPS)

                col_i = cpool.tile([128, 128], F32)
                nc.gpsimd.iota(col_i[:], [[1, 128]], channel_multiplier=0,
                               allow_small_or_imprecise_dtypes=True)
                row_i = cpool.tile([128, 1], F32)
                nc.gpsimd.iota(row_i[:], [[0, 1]], channel_multiplier=1,
                               allow_small_or_imprecise_dtypes=True)
                idn = cpool.tile([128, 128], BF16)
                nc.vector.tensor_scalar(idn[:], col_i[:], row_i[:, 0:1], None,
                                        ALU.is_equal)

            # ---- persistent data tiles ----
                y1 = []
                for qb in range(4):
                    t = dpool.tile([128, D], F32, tag=f"y1_{qb}", name=f"y1_{qb}")
                    nc.sync.dma_start(t[:], xq_d[qb * 128:(qb + 1) * 128, :])
                    y1.append(t)
                W18 = dpool.tile([128, 8, HID], FP8, tag="w18")
                for part in range(4):
                    nc.gpsimd.dma_start(
                        W18[:, 2 * part:2 * part + 2, :],
                        w18_d[:, part * 8192:(part + 1) * 8192]
                        .rearrange("p (j m) -> p j m", j=2))
                w2sb = []
                for j2 in range(32):
                    t = dpool.tile([128, D], BF16, tag=f"w2_{j2}", name=f"w2_{j2}")
                    eng = (nc.sync, nc.gpsimd)[j2 % 2]
                    eng.dma_start(t[:], w2_d[j2 * 128:(j2 + 1) * 128, :])
                    w2sb.append(t)


                # token-major projection first (qa8 ready before any wv;
                # ACT starts packing ~2us in)
                for tb in range(16):
                    ps = qnps.tile([128, D], F32, tag="qn", name=f"qn{tb}")
                    for hh in range(2):
                        for jj in range(4):
                            nc.tensor.matmul(
                                ps[:, hh * 512:(hh + 1) * 512],
                                xT8[jj][:, :, tb * 128:(tb + 1) * 128],
                                wq8[:, 2 * jj:2 * jj + 2,
                                    hh * 512:(hh + 1) * 512],
                                start=(jj == 0), stop=(jj == 3),
                                perf_mode=DR)
                    nc.scalar.copy(
                        qa8[tb // 2][:, tb % 2, :]
                        .rearrange("p (pp hh e) -> p pp hh e", pp=8, hh=2)
                        [:, :, :, 0:64],
                        ps[:].rearrange("p (pp hh e) -> p pp hh e",
                                        pp=8, hh=2))

                for p in range(8):
                    for n in range(4):
                        ps = qpps.tile([128, 512], F32, tag="qp",
                                       name=f"qp{p}_{n}")
                        for jj in range(4):
                            nc.tensor.matmul(
                                ps[:],
                                wq8[:, 2 * jj:2 * jj + 2,
                                    p * 128:(p + 1) * 128],
                                xT8[jj][:, :, n * 512:(n + 1) * 512],
                                start=(jj == 0), stop=(jj == 3),
                                perf_mode=DR)
                        nc.vector.tensor_scalar_add(
                            qT[p][:, n * 512:(n + 1) * 512], ps[:],
                            bqT[:, p:p + 1])

            # ---- phase C: attention with pipelined FFN ----
            with (
                tc.tile_pool(name="cdata", bufs=1) as cd,
                tc.tile_pool(name="scr", bufs=2) as spool,
                tc.tile_pool(name="out1", bufs=1) as opool,
                tc.tile_pool(name="scps", bufs=1, space="PSUM") as scps,
                tc.tile_pool(name="uvps", bufs=2, space="PSUM") as uvps,
                tc.tile_pool(name="fps", bufs=1, space="PSUM") as fps,
                tc.tile_pool(name="w2ps", bufs=1, space="PSUM") as w2ps,
                tc.tile_pool(name="esb", bufs=6) as epool,
            ):
                x1bf = [cd.tile([128, D], BF16, tag=f"x1bf{qb}",
                        name=f"x1bf{qb}") for qb in range(4)]
                x1t8 = cd.tile([128, 8, 256], FP8, tag="x1t8")
                h1all = cd.tile([128, 32, 256], BF16, tag="h1all")
                ps2_live = {}

                def ln_unit(qb):
                    def emit():
                        _layer_norm(nc, spool, y1[qb], x1bf[qb], eps_sb,
                                    (g1bc, be1bc) if apply_affine else None)
                    return emit

                def x1t_unit(qb):
                    def emit():
                        sl = qb % 2
                        with tc.tile_pool(name=f"tp{qb}", bufs=1,
                                          space="PSUM") as tps:
                            for g in range(2):
                                pst = tps.tile([128, 4, 128], BF16,
                                               tag="tps", name=f"tp{qb}_{g}")
                                for k4 in range(4):
                                    k = 4 * g + k4
                                    nc.tensor.transpose(
                                        pst[:, k4, :],
                                        x1bf[qb][:, k * 128:(k + 1) * 128],
                                        idn[:])
                                nc.vector.tensor_copy(
                                    x1t8[:, 4 * g:4 * g + 4,
                                         sl * 128:(sl + 1) * 128], pst[:])
                    return emit

                def w1_unit(qb, g):
                    def emit():
                        sl = qb % 2
                        ps = fps.tile([128, 512], F32, tag="w1p",
                                      name=f"w1_{qb}_{g}")
                        for j4 in range(4):
                            j2 = 4 * g + j4
                            for jj in range(4):
                                nc.tensor.matmul(
                                    ps[:, j4 * 128:(j4 + 1) * 128],
                                    W18[:, 2 * jj:2 * jj + 2,
                                        j2 * 128:(j2 + 1) * 128],
                                    x1t8[:, 2 * jj:2 * jj + 2,
                                         sl * 128:(sl + 1) * 128],
                                    start=(jj == 0), stop=(jj == 3),
                                    perf_mode=DR)
                        for j4 in range(4):
                            nc.vector.tensor_scalar(
                                h1all[:, 4 * g + j4,
                                      sl * 128:(sl + 1) * 128],
                                ps[:, j4 * 128:(j4 + 1) * 128],
                                b1T[:, 4 * g + j4:4 * g + j4 + 1], 0.0,
                                ALU.add, ALU.max)
                    return emit

                def w2_unit(tb, g):
                    def emit():
                        sl = tb % 2
                        if g == 0:
                            ps2_live[tb] = w2ps.tile(
                                [128, D], F32, tag="w2p", name=f"w2ps_{tb}")
                        ps2 = ps2_live[tb]
                        for j4 in range(4):
                            j2 = 4 * g + j4
                            for fc in range(2):
                                nc.tensor.matmul(
                                    ps2[:, fc * 512:(fc + 1) * 512],
                                    h1all[:, j2, sl * 128:(sl + 1) * 128],
                                    w2sb[j2][:, fc * 512:(fc + 1) * 512],
                                    start=(g == 0 and j4 == 0),
                                    stop=(g == 7 and j4 == 3))
                        if g == 7:
                            t1 = y1[tb]        # y1 is dead after LN1
                            nc.vector.tensor_tensor(
                                t1[:], ps2[:], x1bf[tb][:], ALU.add)
                            del ps2_live[tb]
                            y2 = spool.tile([128, D], F32, tag="y2", bufs=1,
                                            name=f"y2_{tb}")
                            nc.gpsimd.tensor_tensor(
                                y2[:], t1[:], b2bc[:], ALU.add)
                            x2 = opool.tile([128, D], F32, tag="x2",
                                            name=f"x2_{tb}")
                            _layer_norm(nc, spool, y2, x2, eps_sb,
                                        (g2bc, be2bc) if apply_affine
                                        else None)
                            nc.sync.dma_start(
                                out_q[tb * 128:(tb + 1) * 128, :], x2[:])
                    return emit

                ffn_fifo = []   # [(earliest_pair_slot, emit_fn)]
                cur_slot = [0]

                def fill():
                    # pop only units whose producers have had >= 2 pairs of
                    # slack, so filler deps never park the in-order PE queue
                    if ffn_fifo and ffn_fifo[0][0] <= cur_slot[0]:
                        ffn_fifo.pop(0)[1]()

                def attn_pair(qb, p):
                    # scores+exp chunk by chunk, FFN filler between chunks
                    # so the in-order PE never parks on the sc psum WAR
                    E_subs = []
                    for sub in range(4):
                        half, kk = sub // 2, sub % 2
                        sc = scps.tile([128, 1024], F32, tag="sc",
                                       name=f"sc{qb}_{p}_{sub}")
                        for kbl in range(8):
                            kb = kk * 8 + kbl
                            nc.tensor.matmul(
                                sc[:, kbl * 128:(kbl + 1) * 128],
                                qT[p][half * 64:half * 64 + 64,
                                      kb * 128:(kb + 1) * 128],
                                qT[p][half * 64:half * 64 + 64,
                                      qb * 128:(qb + 1) * 128],
                                start=True, stop=True)
                        E = epool.tile([128, 1024], FP8, tag="E",
                                       name=f"E{qb}_{p}_{sub}")
                        nc.scalar.activation(E[:], sc[:], AF.Exp,
                                             scale=0.125)
                        E_subs.append(E)
                        fill()
                    uv = uvps.tile([128, 130], F32, tag="uv",
                                   name=f"uv{qb}_{p}")
                    for half in range(2):
                        off = p * 130 + half * 65
                        for c in range(8):
                            E = E_subs[half * 2 + c // 4]
                            cl = c % 4
                            nc.tensor.matmul(
                                uv[:, half * 65:half * 65 + 65],
                                E[:].rearrange("p (j q) -> p j q", j=8)
                                [:, 2 * cl:2 * cl + 2, :],
                                qa8[c][:, :, off:off + 65],
                                start=(c == 0), stop=(c == 7),
                                perf_mode=DR)
                    for half in range(2):
                        h = 2 * p + half
                        rcp = spool.tile([128, 1], F32, tag="rcp",
                                         bufs=4, name=f"rcp{qb}_{h}")
                        nc.vector.reciprocal(
                            rcp[:], uv[:, half * 65 + 64:half * 65 + 65])
                        nc.vector.scalar_tensor_tensor(
                            y1[qb][:, h * 64:(h + 1) * 64],
                            uv[:, half * 65:half * 65 + 64],
                            rcp[:, 0:1],
                            y1[qb][:, h * 64:(h + 1) * 64],
                            ALU.mult, ALU.add)
                    fill()

                for qb in range(4):
                    for p in range(8):
                        cur_slot[0] = qb * 8 + p
                        attn_pair(qb, p)
                    last = qb * 8 + 7
                    ffn_fifo.append((last + 2, ln_unit(qb)))
                    ffn_fifo.append((last + 3, x1t_unit(qb)))
                    for g in range(8):
                        ffn_fifo.append((last + 4 + g // 4, w1_unit(qb, g)))
                    for g in range(8):
                        ffn_fifo.append((last + 5 + g // 2, w2_unit(qb, g)))
                while ffn_fifo:
                    ffn_fifo.pop(0)[1]()

    nc.compile()
    _BUILD_CACHE[apply_affine] = nc
    return nc


def _layer_norm(nc, pool, y, out, eps_sb, affine):
    """out = (y - mean) * rsqrt(var + EPS) [* g + b]; free-dim D, f32 in.

    The tensor_tensor_reduce product output is junk scratch; it is written
    into `out`, which is then overwritten by the real normalized value.
    """
    s1 = pool.tile([128, 1], F32, tag="ln_s1")
    nc.vector.reduce_sum(s1[:], y[:], axis=mybir.AxisListType.X)
    mean = pool.tile([128, 1], F32, tag="ln_mean")
    nc.vector.tensor_scalar_mul(mean[:], s1[:], 1.0 / D)
    sqs = pool.tile([128, 1], F32, tag="ln_sqs")
    nc.vector.tensor_tensor_reduce(out[:], y[:], y[:], 1.0 / D, 0.0,
                                   ALU.mult, ALU.add, sqs[:])
    msq = pool.tile([128, 1], F32, tag="ln_msq")
    nc.vector.tensor_tensor(msq[:], mean[:], mean[:], ALU.mult)
    var = pool.tile([128, 1], F32, tag="ln_var")
    nc.vector.tensor_tensor(var[:], sqs[:], msq[:], ALU.subtract)
    lnv = pool.tile([128, 1], F32, tag="ln_lnv")
    nc.scalar.activation(lnv[:], var[:], AF.Ln, bias=eps_sb[:, 0:1])
    rstd = pool.tile([128, 1], F32, tag="ln_rstd")
    nc.scalar.activation(rstd[:], lnv[:], AF.Exp, scale=-0.5)
    if affine is None:
        nc.vector.tensor_scalar(out[:], y[:], mean[:, 0:1], rstd[:, 0:1],
                                ALU.subtract, ALU.mult)
    else:
        g_bc, b_bc = affine
        nc.vector.tensor_scalar(out[:], y[:], mean[:, 0:1], rstd[:, 0:1],
                                ALU.subtract, ALU.mult)
        nc.vector.tensor_tensor(out[:], out[:], g_bc[:], ALU.mult)
        nc.vector.tensor_tensor(out[:], out[:], b_bc[:], ALU.add)


def kernel(x, Wq, bq, ln1_g, ln1_b, W1, b1, W2, b2, ln2_g, ln2_b):
    x = np.asarray(x, np.float32)
    bf = dt.np(BF16)
    f8 = dt.np(FP8)
    trivial = (np.all(ln1_g == 1) and np.all(ln1_b == 0)
               and np.all(ln2_g == 1) and np.all(ln2_b == 0))
    nc = _build(apply_affine=not trivial)

    WqF = np.asarray(Wq, np.float32).transpose(1, 0, 2).reshape(D, D)
    bqF = np.asarray(bq, np.float32).reshape(D)
    W1f = np.asarray(W1, np.float32)

    wq8 = np.ascontiguousarray(
        WqF.astype(f8).reshape(8, 128, D).transpose(1, 0, 2)
        .reshape(128, 8 * D))
    bqT = np.ascontiguousarray(bqF.reshape(8, 128).T)
    w18 = np.ascontiguousarray(
        W1f.astype(f8).reshape(8, 128, HID).transpose(1, 0, 2)
        .reshape(128, 8 * HID))
    b1T = np.ascontiguousarray(
        np.asarray(b1, np.float32).reshape(32, 128).T)
    w2bf = np.asarray(W2, np.float32).astype(bf)
    b2bc = np.ascontiguousarray(
        np.broadcast_to(np.asarray(b2, np.float32), (128, D)))

    base = {"wq8": wq8, "bqT": bqT, "w18": w18, "b1T": b1T,
            "w2": w2bf, "b2bc": b2bc}
    if not trivial:
        for name, v in (("g1bc", ln1_g), ("be1bc", ln1_b),
                        ("g2bc", ln2_g), ("be2bc", ln2_b)):
            base[name] = np.ascontiguousarray(
                np.broadcast_to(np.asarray(v, np.float32), (128, D)))

    in_maps = []
    for c in range(NCORES):
        b, t = divmod(c, 4)
        xb = np.concatenate([x[b, t * SQ:], x[b, :t * SQ]], axis=0)
        xt8 = np.ascontiguousarray(
            xb.T.astype(f8).reshape(4, 2, 128, S).transpose(2, 0, 1, 3)
            .reshape(128, 16384))
        in_maps.append({
            **base,
            "xt8": xt8,
            "xq": np.ascontiguousarray(xb[:SQ] + bqF[None, :]),
        })

    import os
    trace = bool(int(os.environ.get("KERNEL_TRACE", "0")))
    kw = {}
    if trace:
        kw = dict(trace=True,
                  tmpdir=os.environ.get("KERNEL_TRACE_DIR") or None)
    res = run_bass_kernel_spmd(nc, in_maps, core_ids=list(range(NCORES)),
                               **kw)
    if trace:
        print(f"HW exec time: {res.exec_time_ns} ns")
    out = np.empty((B, S, D), np.float32)
    for c in range(NCORES):
        b, t = divmod(c, 4)
        out[b, t * SQ:(t + 1) * SQ] = res.results[c]["out_q"]
    return out


# revision 13
# speedup vs baseline: 1.5415x; 1.3576x over previous
"""Trainium2 Bass kernel for a dense transformer encoder layer (v2).

Reference semantics (B=2, S=2048, D=1024, H=16, DH=64, HID=4096):
    q = einsum('bsd,hde->bhse', x, Wq) + bq          (q == k == v, source bug)
    prob = softmax(q @ q^T / sqrt(DH))
    attn = concat_heads(prob @ q)
    x1 = LN(x + attn);  ff = relu(x1 @ W1 + b1) @ W2 + b2;  out = LN(x1 + ff)

Sharding: 8 cores, core c -> batch b=c//4, token quarter t=c%4.  The host
rotates x[b] so the core's 512 queries are tokens 0:511 (attention is
permutation-equivariant over keys); q/k/v for the full sequence are computed
on every core of the 4-core group (zero collectives).

v2 design (driven by the TimelineSim cost model):
- fp8e4m3 DoubleRow matmuls (two stacked 128-row K-subtiles per instruction)
  for the q projection, attention@V, and FFN W1.  Scores and W2 stay bf16
  for accuracy.  All weight layouts are prepared host-side; there are no
  DMA transposes and no DRAM round-trips for intermediates.
- q is computed in BOTH layouts directly on the PE: feature-major qT (bf16,
  for scores) and token-major qa8 (fp8, for attention@V), reusing the same
  x^T / Wq operand tensors in swapped stationary/moving roles.
- bq is folded out of the value path: softmax weights sum to 1, so
  attn = P@(x Wq) + bq, and the +bq folds into the host residual x_q=x+bq.
  qT keeps the bias (scores need it).
- attention@V emits token-major [128q, 65] psum tiles (64 head features +
  the softmax denominator from an all-ones column of qa8); the epilogue is
  a reciprocal + one fused multiply-add into y1 per head - no transposes.
- Softmax exp on ACT is the critical engine (~127us); attention runs
  query-block-major and each block's FFN (token-major outputs too) is
  emitted under the exp shadow via a work-unit FIFO.
"""

import numpy as np

import concourse.bacc as bacc
import concourse.mybir as mybir
from concourse import tile
from concourse.bass_utils import run_bass_kernel_spmd

dt = mybir.dt
AF = mybir.ActivationFunctionType
ALU = mybir.AluOpType
DR = mybir.MatmulPerfMode.DoubleRow

B, S, D = 2, 2048, 1024
H, DH, HID = 16, 64, 4096
SQ = S // 4            # tokens per core
NCORES = 8
EPS = 1e-5
F32, BF16, FP8 = dt.float32, dt.bfloat16, dt.float8e4

_BUILD_CACHE = {}


def _build(apply_affine: bool):
    if apply_affine in _BUILD_CACHE:
        return _BUILD_CACHE[apply_affine]

    nc = bacc.Bacc("TRN2", target_bir_lowering=False, debug=False,
                   num_devices=NCORES)

    xt8_d = nc.dram_tensor("xt8", [128, 16384], FP8, kind="ExternalInput").ap()
    wq8_d = nc.dram_tensor("wq8", [128, 8192], FP8, kind="ExternalInput").ap()
    bqT_d = nc.dram_tensor("bqT", [128, 8], F32, kind="ExternalInput").ap()
    xq_d = nc.dram_tensor("xq", [SQ, D], F32, kind="ExternalInput").ap()
    w18_d = nc.dram_tensor("w18", [128, 32768], FP8,
                           kind="ExternalInput").ap()
    b1T_d = nc.dram_tensor("b1T", [128, 32], F32, kind="ExternalInput").ap()
    w2_d = nc.dram_tensor("w2", [HID, D], BF16, kind="ExternalInput").ap()
    b2bc_d = nc.dram_tensor("b2bc", [128, D], F32, kind="ExternalInput").ap()
    if apply_affine:
        g1_d = nc.dram_tensor("g1bc", [128, D], F32, kind="ExternalInput").ap()
        be1_d = nc.dram_tensor("be1bc", [128, D], F32,
                               kind="ExternalInput").ap()
        g2_d = nc.dram_tensor("g2bc", [128, D], F32, kind="ExternalInput").ap()
        be2_d = nc.dram_tensor("be2bc", [128, D], F32,
                               kind="ExternalInput").ap()
    out_q = nc.dram_tensor("out_q", [SQ, D], F32, kind="ExternalOutput").ap()

    with tile.TileContext(nc) as tc:
        with (
            tc.tile_pool(name="const", bufs=1) as cpool,
            tc.tile_pool(name="data", bufs=1) as dpool,
        ):
            qT = [dpool.tile([128, S], BF16, tag=f"qT{p}", name=f"qT{p}")
                  for p in range(8)]
            # qa8[c][p, j, pp*130 + hh*65 + e] = q_nat[key 128*(2c+j)+p,
            #   feature pp*128+hh*64+e]; column e=64 of each 65-block is 1.0
            qa8 = [dpool.tile([128, 2, 1040], FP8, tag=f"qa8_{c}",
                              name=f"qa8_{c}") for c in range(8)]
            for c in range(8):
                nc.gpsimd.memset(
                    qa8[c][:].rearrange("p j (pp hh e) -> p (j pp hh) e",
                                        pp=8, hh=2)[:, :, 64:65], 1.0)

            # ---- phase B: q projection, both layouts ----
            with (
                tc.tile_pool(name="bload", bufs=1) as bpool,
                tc.tile_pool(name="qpps", bufs=2, space="PSUM") as qpps,
                tc.tile_pool(name="qnps", bufs=2, space="PSUM") as qnps,
            ):
                xT8 = []
                for jj in range(4):
                    t = bpool.tile([128, 2, S], FP8, tag=f"xT8_{jj}", name=f"xT8_{jj}")
                    nc.sync.dma_start(
                        t[:], xt8_d[:, jj * 4096:(jj + 1) * 4096]
                        .rearrange("p (j t) -> p j t", j=2))
                    xT8.append(t)
                wq8 = bpool.tile([128, 8, D], FP8, tag="wq8")
                nc.sync.dma_start(
                    wq8[:], wq8_d.rearrange("p (j m) -> p j m", j=8))

                # ---- constants ----
                bqT = cpool.tile([128, 8], F32)
                nc.sync.dma_start(bqT[:], bqT_d[:])
                b1T = cpool.tile([128, 32], F32)
                nc.sync.dma_start(b1T[:], b1T_d[:])
                b2bc = cpool.tile([128, D], F32)
                nc.gpsimd.dma_start(b2bc[:], b2bc_d[:])
                if apply_affine:
                    g1bc = cpool.tile([128, D], F32)
                    nc.gpsimd.dma_start(g1bc[:], g1_d[:])
                    be1bc = cpool.tile([128, D], F32)
                    nc.gpsimd.dma_start(be1bc[:], be1_d[:])
                    g2bc = cpool.tile([128, D], F32)
                    nc.gpsimd.dma_start(g2bc[:], g2_d[:])
                    be2bc = cpool.tile([128, D], F32)
                    nc.gpsimd.dma_start(be2bc[:], be2_d[:])
                eps_sb = cpool.tile([128, 1], F32)
                nc.vector.memset(eps_sb[:], EPS)

                col_i = cpool.tile([128, 128], F32)
                nc.gpsimd.iota(col_i[:], [[1, 128]], channel_multiplier=0,
                               allow_small_or_imprecise_dtypes=True)
                row_i = cpool.tile([128, 1], F32)
                nc.gpsimd.iota(row_i[:], [[0, 1]], channel_multiplier=1,
                               allow_small_or_imprecise_dtypes=True)
                idn = cpool.tile([128, 128], BF16)
                nc.vector.tensor_scalar(idn[:], col_i[:], row_i[:, 0:1], None,
                                        ALU.is_equal)

            # ---- persistent data tiles ----
                y1 = []
                for qb in range(4):
                    t = dpool.tile([128, D], F32, tag=f"y1_{qb}", name=f"y1_{qb}")
                    nc.sync.dma_start(t[:], xq_d[qb * 128:(qb + 1) * 128, :])
                    y1.append(t)
                W18 = dpool.tile([128, 8, HID], FP8, tag="w18")
                for part in range(4):
                    nc.gpsimd.dma_start(
                        W18[:, 2 * part:2 * part + 2, :],
                        w18_d[:, part * 8192:(part + 1) * 8192]
                        .rearrange("p (j m) -> p j m", j=2))
                w2sb = []
                for j2 in range(32):
                    t = dpool.tile([128, D], BF16, tag=f"w2_{j2}", name=f"w2_{j2}")
                    eng = (nc.sync, nc.gpsimd)[j2 % 2]
                    eng.dma_start(t[:], w2_d[j2 * 128:(j2 + 1) * 128, :])
                    w2sb.append(t)


                # token-major projection first (qa8 ready before any wv;
                # ACT starts packing ~2us in)
                for tb in range(16):
                    ps = qnps.tile([128, D], F32, tag="qn", name=f"qn{tb}")
                    for hh in range(2):
                        for jj in range(4):
                            nc.tensor.matmul(
                                ps[:, hh * 512:(hh + 1) * 512],
                                xT8[jj][:, :, tb * 128:(tb + 1) * 128],
                                wq8[:, 2 * jj:2 * jj + 2,
                                    hh * 512:(hh + 1) * 512],
                                start=(jj == 0), stop=(jj == 3),
                                perf_mode=DR)
                    nc.scalar.copy(
                        qa8[tb // 2][:, tb % 2, :]
                        .rearrange("p (pp hh e) -> p pp hh e", pp=8, hh=2)
                        [:, :, :, 0:64],
                        ps[:].rearrange("p (pp hh e) -> p pp hh e",
                                        pp=8, hh=2))

                for p in range(8):
                    for n in range(4):
                        ps = qpps.tile([128, 512], F32, tag="qp",
                                       name=f"qp{p}_{n}")
                        for jj in range(4):
                            nc.tensor.matmul(
                                ps[:],
                                wq8[:, 2 * jj:2 * jj + 2,
                                    p * 128:(p + 1) * 128],
                                xT8[jj][:, :, n * 512:(n + 1) * 512],
                                start=(jj == 0), stop=(jj == 3),
                                perf_mode=DR)
                        nc.vector.tensor_scalar_add(
                            qT[p][:, n * 512:(n + 1) * 512], ps[:],
                            bqT[:, p:p + 1])

            # ---- phase C: attention with pipelined FFN ----
            with (
                tc.tile_pool(name="cdata", bufs=1) as cd,
                tc.tile_pool(name="scr", bufs=2) as spool,
                tc.tile_pool(name="out1", bufs=1) as opool,
                tc.tile_pool(name="scps", bufs=2, space="PSUM") as scps,
                tc.tile_pool(name="uvps", bufs=1, space="PSUM") as uvps,
                tc.tile_pool(name="fps", bufs=1, space="PSUM") as fps,
                tc.tile_pool(name="w2ps", bufs=1, space="PSUM") as w2ps,
                tc.tile_pool(name="esb", bufs=6) as epool,
            ):
                x1bf = [cd.tile([128, D], BF16, tag=f"x1bf{qb}",
                        name=f"x1bf{qb}") for qb in range(4)]
                x1t8 = cd.tile([128, 8, 256], FP8, tag="x1t8")
                h1all = cd.tile([128, 32, 256], BF16, tag="h1all")
                ps2_live = {}

                def ln_unit(qb):
                    def emit():
                        _layer_norm(nc, spool, y1[qb], x1bf[qb], eps_sb,
                                    (g1bc, be1bc) if apply_affine else None)
                    return emit

                def x1t_unit(qb):
                    def emit():
                        sl = qb % 2
                        with tc.tile_pool(name=f"tp{qb}", bufs=1,
                                          space="PSUM") as tps:
                            for g in range(2):
                                pst = tps.tile([128, 4, 128], BF16,
                                               tag="tps", name=f"tp{qb}_{g}")
                                for k4 in range(4):
                                    k = 4 * g + k4
                                    nc.tensor.transpose(
                                        pst[:, k4, :],
                                        x1bf[qb][:, k * 128:(k + 1) * 128],
                                        idn[:])
                                nc.vector.tensor_copy(
                                    x1t8[:, 4 * g:4 * g + 4,
                                         sl * 128:(sl + 1) * 128], pst[:])
                    return emit

                def w1_unit(qb, g):
                    def emit():
                        sl = qb % 2
                        ps = fps.tile([128, 512], F32, tag="w1p",
                                      name=f"w1_{qb}_{g}")
                        for j4 in range(4):
                            j2 = 4 * g + j4
                            for jj in range(4):
                                nc.tensor.matmul(
                                    ps[:, j4 * 128:(j4 + 1) * 128],
                                    W18[:, 2 * jj:2 * jj + 2,
                                        j2 * 128:(j2 + 1) * 128],
                                    x1t8[:, 2 * jj:2 * jj + 2,
                                         sl * 128:(sl + 1) * 128],
                                    start=(jj == 0), stop=(jj == 3),
                                    perf_mode=DR)
                        for j4 in range(4):
                            nc.vector.tensor_scalar(
                                h1all[:, 4 * g + j4,
                                      sl * 128:(sl + 1) * 128],
                                ps[:, j4 * 128:(j4 + 1) * 128],
                                b1T[:, 4 * g + j4:4 * g + j4 + 1], 0.0,
                                ALU.add, ALU.max)
                    return emit

                def w2_unit(tb, fc, g):
                    def emit():
                        sl = tb % 2
                        if g == 0:
                            ps2_live[tb] = w2ps.tile(
                                [128, 512], F32, tag="w2p",
                                name=f"w2ps_{tb}_{fc}")
                        ps2 = ps2_live[tb]
                        for j4 in range(4):
                            j2 = 4 * g + j4
                            nc.tensor.matmul(
                                ps2[:],
                                h1all[:, j2, sl * 128:(sl + 1) * 128],
                                w2sb[j2][:, fc * 512:(fc + 1) * 512],
                                start=(g == 0 and j4 == 0),
                                stop=(g == 7 and j4 == 3))
                        if g == 7:
                            t1 = y1[tb]        # y1 is dead after LN1
                            nc.vector.tensor_tensor(
                                t1[:, fc * 512:(fc + 1) * 512], ps2[:],
                                x1bf[tb][:, fc * 512:(fc + 1) * 512],
                                ALU.add)
                            del ps2_live[tb]
                        if fc == 1 and g == 7:
                            y2 = spool.tile([128, D], F32, tag="y2", bufs=1,
                                            name=f"y2_{tb}")
                            nc.gpsimd.tensor_tensor(
                                y2[:], t1[:], b2bc[:], ALU.add)
                            x2 = opool.tile([128, D], F32, tag="x2",
                                            name=f"x2_{tb}")
                            _layer_norm(nc, spool, y2, x2, eps_sb,
                                        (g2bc, be2bc) if apply_affine
                                        else None)
                            nc.sync.dma_start(
                                out_q[tb * 128:(tb + 1) * 128, :], x2[:])
                    return emit

                ffn_fifo = []   # [(earliest_pair_slot, emit_fn)]
                cur_slot = [0]

                def fill():
                    # pop only units whose producers have had >= 2 pairs of
                    # slack, so filler deps never park the in-order PE queue
                    if ffn_fifo and ffn_fifo[0][0] <= cur_slot[0]:
                        ffn_fifo.pop(0)[1]()

                def attn_pair(qb, p):
                    # scores+exp chunk by chunk, FFN filler between chunks
                    # so the in-order PE never parks on the sc psum WAR
                    E_subs = []
                    for sub in range(4):
                        half, kk = sub // 2, sub % 2
                        sc = scps.tile([128, 1024], F32, tag="sc",
                                       name=f"sc{qb}_{p}_{sub}")
                        for kbl in range(8):
                            kb = kk * 8 + kbl
                            nc.tensor.matmul(
                                sc[:, kbl * 128:(kbl + 1) * 128],
                                qT[p][half * 64:half * 64 + 64,
                                      kb * 128:(kb + 1) * 128],
                                qT[p][half * 64:half * 64 + 64,
                                      qb * 128:(qb + 1) * 128],
                                start=True, stop=True)
                        E = epool.tile([128, 1024], FP8, tag="E",
                                       name=f"E{qb}_{p}_{sub}")
                        nc.scalar.activation(E[:], sc[:], AF.Exp,
                                             scale=0.125)
                        E_subs.append(E)
                        fill()
                    uv = uvps.tile([128, 130], F32, tag="uv",
                                   name=f"uv{qb}_{p}")
                    for half in range(2):
                        off = p * 130 + half * 65
                        for c in range(8):
                            E = E_subs[half * 2 + c // 4]
                            cl = c % 4
                            nc.tensor.matmul(
                                uv[:, half * 65:half * 65 + 65],
                                E[:].rearrange("p (j q) -> p j q", j=8)
                                [:, 2 * cl:2 * cl + 2, :],
                                qa8[c][:, :, off:off + 65],
                                start=(c == 0), stop=(c == 7),
                                perf_mode=DR)
                    for half in range(2):
                        h = 2 * p + half
                        rcp = spool.tile([128, 1], F32, tag="rcp",
                                         bufs=4, name=f"rcp{qb}_{h}")
                        nc.vector.reciprocal(
                            rcp[:], uv[:, half * 65 + 64:half * 65 + 65])
                        nc.vector.scalar_tensor_tensor(
                            y1[qb][:, h * 64:(h + 1) * 64],
                            uv[:, half * 65:half * 65 + 64],
                            rcp[:, 0:1],
                            y1[qb][:, h * 64:(h + 1) * 64],
                            ALU.mult, ALU.add)
                    fill()

                for qb in range(4):
                    for p in range(8):
                        cur_slot[0] = qb * 8 + p
                        attn_pair(qb, p)
                    last = qb * 8 + 7
                    ffn_fifo.append((last + 2, ln_unit(qb)))
                    ffn_fifo.append((last + 3, x1t_unit(qb)))
                    for g in range(8):
                        ffn_fifo.append((last + 4 + g // 4, w1_unit(qb, g)))
                    for fc in range(2):
                        for g in range(8):
                            ffn_fifo.append(
                                (last + 5 + (fc * 8 + g) // 4,
                                 w2_unit(qb, fc, g)))
                while ffn_fifo:
                    ffn_fifo.pop(0)[1]()

    import concourse.bacc as _bacc_mod
    _orig_tables = _bacc_mod.get_activation_tables

    def _reordered(arch):
        t = dict(_orig_tables(arch))
        pref = [k for k in t if "natural_log_exp" in k]
        if pref:
            t = {**{k: t[k] for k in pref},
                 **{k: v for k, v in t.items() if k not in pref}}
        return t

    _bacc_mod.get_activation_tables = _reordered
    try:
        nc.compile()
    finally:
        _bacc_mod.get_activation_tables = _orig_tables
    _BUILD_CACHE[apply_affine] = nc
    return nc


def _layer_norm(nc, pool, y, out, eps_sb, affine):
    """out = (y - mean) * rsqrt(var + EPS) [* g + b]; free-dim D, f32 in.

    The tensor_tensor_reduce product output is junk scratch; it is written
    into `out`, which is then overwritten by the real normalized value.
    """
    s1 = pool.tile([128, 1], F32, tag="ln_s1")
    nc.vector.reduce_sum(s1[:], y[:], axis=mybir.AxisListType.X)
    mean = pool.tile([128, 1], F32, tag="ln_mean")
    nc.vector.tensor_scalar_mul(mean[:], s1[:], 1.0 / D)
    sqs = pool.tile([128, 1], F32, tag="ln_sqs")
    nc.vector.tensor_tensor_reduce(out[:], y[:], y[:], 1.0 / D, 0.0,
                                   ALU.mult, ALU.add, sqs[:])
    msq = pool.tile([128, 1], F32, tag="ln_msq")
    nc.vector.tensor_tensor(msq[:], mean[:], mean[:], ALU.mult)
    var = pool.tile([128, 1], F32, tag="ln_var")
    nc.vector.tensor_tensor(var[:], sqs[:], msq[:], ALU.subtract)
    lnv = pool.tile([128, 1], F32, tag="ln_lnv")
    nc.scalar.activation(lnv[:], var[:], AF.Ln, bias=eps_sb[:, 0:1])
    rstd = pool.tile([128, 1], F32, tag="ln_rstd")
    nc.scalar.activation(rstd[:], lnv[:], AF.Exp, scale=-0.5)
    if affine is None:
        nc.vector.tensor_scalar(out[:], y[:], mean[:, 0:1], rstd[:, 0:1],
                                ALU.subtract, ALU.mult)
    else:
        g_bc, b_bc = affine
        nc.vector.tensor_scalar(out[:], y[:], mean[:, 0:1], rstd[:, 0:1],
                                ALU.subtract, ALU.mult)
        nc.vector.tensor_tensor(out[:], out[:], g_bc[:], ALU.mult)
        nc.vector.tensor_tensor(out[:], out[:], b_bc[:], ALU.add)


def kernel(x, Wq, bq, ln1_g, ln1_b, W1, b1, W2, b2, ln2_g, ln2_b):
    x = np.asarray(x, np.float32)
    bf = dt.np(BF16)
    f8 = dt.np(FP8)
    trivial = (np.all(ln1_g == 1) and np.all(ln1_b == 0)
               and np.all(ln2_g == 1) and np.all(ln2_b == 0))
    nc = _build(apply_affine=not trivial)

    WqF = np.asarray(Wq, np.float32).transpose(1, 0, 2).reshape(D, D)
    bqF = np.asarray(bq, np.float32).reshape(D)
    W1f = np.asarray(W1, np.float32)

    wq8 = np.ascontiguousarray(
        WqF.astype(f8).reshape(8, 128, D).transpose(1, 0, 2)
        .reshape(128, 8 * D))
    bqT = np.ascontiguousarray(bqF.reshape(8, 128).T)
    w18 = np.ascontiguousarray(
        W1f.astype(f8).reshape(8, 128, HID).transpose(1, 0, 2)
        .reshape(128, 8 * HID))
    b1T = np.ascontiguousarray(
        np.asarray(b1, np.float32).reshape(32, 128).T)
    w2bf = np.asarray(W2, np.float32).astype(bf)
    b2bc = np.ascontiguousarray(
        np.broadcast_to(np.asarray(b2, np.float32), (128, D)))

    base = {"wq8": wq8, "bqT": bqT, "w18": w18, "b1T": b1T,
            "w2": w2bf, "b2bc": b2bc}
    if not trivial:
        for name, v in (("g1bc", ln1_g), ("be1bc", ln1_b),
                        ("g2bc", ln2_g), ("be2bc", ln2_b)):
            base[name] = np.ascontiguousarray(
                np.broadcast_to(np.asarray(v, np.float32), (128, D)))

    in_maps = []
    for c in range(NCORES):
        b, t = divmod(c, 4)
        xb = np.concatenate([x[b, t * SQ:], x[b, :t * SQ]], axis=0)
        xt8 = np.ascontiguousarray(
            xb.T.astype(f8).reshape(4, 2, 128, S).transpose(2, 0, 1, 3)
            .reshape(128, 16384))
        in_maps.append({
            **base,
            "xt8": xt8,
            "xq": np.ascontiguousarray(xb[:SQ] + bqF[None, :]),
        })

    import os
    trace = bool(int(os.environ.get("KERNEL_TRACE", "0")))
    kw = {}
    if trace:
        kw = dict(trace=True,
                  tmpdir=os.environ.get("KERNEL_TRACE_DIR") or None)
    res = run_bass_kernel_spmd(nc, in_maps, core_ids=list(range(NCORES)),
                               **kw)
    if trace:
        print(f"HW exec time: {res.exec_time_ns} ns")
    out = np.empty((B, S, D), np.float32)
    for c in range(NCORES):
        b, t = divmod(c, 4)
        out[b, t * SQ:(t + 1) * SQ] = res.results[c]["out_q"]
    return out
